# revision 30
# baseline (speedup 1.0000x reference)
import sys

sys.path.insert(0, "/opt/trn_rl_repo")

from contextlib import ExitStack

import numpy as np

P, HO, WO = 7, 8, 32
N_ROIS = 512
NCORES = 8
SIZES = (256, 128, 64, 32)
PIPE = 6
PIPE_O = 12
CH = 4
NBUF = 6
IDX16_MAX = 32766

_TRACE = False
LAST_EXEC_NS = None


def _grid_and_levels(polys):
    import jax
    import jax.numpy as jnp

    cpu = jax.devices("cpu")[0]
    with jax.default_device(cpu):
        pj = jnp.asarray(np.asarray(polys), jnp.float32)
        x, y = pj[..., 0], pj[..., 1]
        area = 0.5 * jnp.abs(
            jnp.sum(x * jnp.roll(y, -1, axis=1) - jnp.roll(x, -1, axis=1) * y, axis=1)
        )
        s = jnp.sqrt(area)
        lvls = (
            jnp.clip(jnp.floor(4.0 + jnp.log2(s / 224.0 + 1e-6)), 2, 5).astype(jnp.int32)
            - 2
        )
        idx = np.concatenate([np.arange(P), np.arange(2 * P - 1, P - 1, -1)])
        pp = pj[:, idx]
        wh = jnp.array([1024.0, 1024.0], jnp.float32)
        pn = pp / wh
        top, bot = pn[:, :P], pn[:, P:]
        u = jnp.linspace(0.0, P - 1.0, WO)
        i0 = jnp.clip(jnp.floor(u).astype(jnp.int32), 0, P - 2)
        f = (u - i0)[:, None]
        topw = top[:, i0] * (1 - f) + top[:, i0 + 1] * f
        botw = bot[:, i0] * (1 - f) + bot[:, i0 + 1] * f
        tt = jnp.linspace(0.0, 1.0, HO)[None, :, None, None]
        grid = (1 - tt) * topw[:, None] + tt * botw[:, None]  # [N,HO,WO,2]
        grid_np = np.asarray(jax.device_get(grid), np.float32)
        lvls_np = np.asarray(jax.device_get(lvls), np.int32)
    return grid_np, lvls_np


def _corners(grid_np, lvls_np, img_ids):
    ids = np.asarray(img_ids).astype(np.int64)
    n = grid_np.shape[0]
    npts = HO * WO
    seg0 = np.empty((n, npts), np.int64)
    w00 = np.empty((n, npts), np.float32)
    w01 = np.empty((n, npts), np.float32)
    w10 = np.empty((n, npts), np.float32)
    w11 = np.empty((n, npts), np.float32)
    for lev, S in enumerate(SIZES):
        m = lvls_np == lev
        if not m.any():
            continue
        g = grid_np[m]
        sf = np.float32(S - 1)
        xs = np.clip(g[..., 0] * sf, np.float32(0.0), sf)
        ys = np.clip(g[..., 1] * sf, np.float32(0.0), sf)
        x0 = np.minimum(np.floor(xs), np.float32(S - 2))
        y0 = np.minimum(np.floor(ys), np.float32(S - 2))
        fx = xs - x0
        fy = ys - y0
        x0i = x0.astype(np.int64)
        y0i = y0.astype(np.int64)
        b = ids[m][:, None, None]
        sg = (b * S + y0i) * S + x0i
        seg0[m] = sg.reshape(-1, npts)
        w00[m] = ((1 - fx) * (1 - fy)).reshape(-1, npts)
        w01[m] = (fx * (1 - fy)).reshape(-1, npts)
        w10[m] = ((1 - fx) * fy).reshape(-1, npts)
        w11[m] = (fx * fy).reshape(-1, npts)
    return seg0, (w00, w01, w10, w11)


def _build_groups(seg0, lvls_np):
    groups = []  # (lvl, base, nrows, member_roi_list)
    idx0 = np.where(lvls_np == 0)[0]
    if len(idx0):
        S = SIZES[0]
        TOT = 2 * S * S
        segmin = seg0[idx0].min(axis=1)
        segmax = seg0[idx0].max(axis=1)
        order = np.argsort(segmin, kind="stable")
        base = None
        cur = []
        for j in order:
            r = int(idx0[j])
            if base is None:
                base, cur = int(segmin[j]), [r]
            elif int(segmax[j]) - base <= IDX16_MAX:
                cur.append(r)
            else:
                groups.append((0, base, min(32767, TOT - base), cur))
                base, cur = int(segmin[j]), [r]
        groups.append((0, base, min(32767, TOT - base), cur))
    for lev in (1, 2, 3):
        rois = [int(r) for r in np.where(lvls_np == lev)[0]]
        if rois:
            TOT = 2 * SIZES[lev] * SIZES[lev]
            groups.append((lev, 0, min(32767, TOT), rois))
    return groups


def _deal(groups):
    slot_groups = []  # (lvl, base, nrows) per slot, identical across cores
    core_slots = [[] for _ in range(NCORES)]  # per core: (roi, is_dummy)
    for lvl, base, nrows, members in groups:
        pad = (-len(members)) % NCORES
        mem = members + [-1] * pad
        nslots = len(mem) // NCORES
        for t in range(nslots):
            slot_groups.append((lvl, base, nrows))
            for c in range(NCORES):
                m = mem[t * NCORES + c]
                if m < 0:
                    core_slots[c].append((members[0], True))
                else:
                    core_slots[c].append((m, False))
    return slot_groups, core_slots


def _build_core_inputs(slot_groups, core_slots, seg0, weights):
    nslots = len(slot_groups)
    w00, w01, w10, w11 = weights
    idx16 = np.zeros((NCORES, 128, nslots * 16), np.int16)
    wts = np.zeros((NCORES, 128, nslots * 8), np.float32)
    for c in range(NCORES):
        for s, (roi, _dummy) in enumerate(core_slots[c]):
            lvl, base, nrows = slot_groups[s]
            q = seg0[roi] - base
            assert q.min() >= 0 and q.max() < nrows and q.max() <= IDX16_MAX, (
                c, s, lvl, base, nrows, int(q.min()), int(q.max()))
            t16 = q.reshape(16, 16).T.astype(np.int16)
            idx16[c, :, s * 16 : (s + 1) * 16] = np.tile(t16, (8, 1))
            for b in (0, 1):
                sl = slice(b * 128, (b + 1) * 128)
                col = s * 8 + b * 4
                wts[c, :, col + 0] = w00[roi][sl]
                wts[c, :, col + 1] = w01[roi][sl]
                wts[c, :, col + 2] = w10[roi][sl]
                wts[c, :, col + 3] = w11[roi][sl]
    return idx16, wts


def _chunks(slot_groups):
    # runs of identical (lvl, base, nrows), capped at CH slots per gather
    chunks = []  # (lvl, base, nrows, s0, k)
    s = 0
    n = len(slot_groups)
    while s < n:
        lvl, base, nrows = slot_groups[s]
        k = 1
        while s + k < n and k < CH and slot_groups[s + k] == (lvl, base, nrows):
            k += 1
        chunks.append((lvl, base, nrows, s, k))
        s += k
    return chunks


def _build_device(slot_groups):
    import concourse.bacc as bacc
    import concourse.bass as bass
    import concourse.mybir as mybir
    from concourse import library_config

    f32, f16, i16 = mybir.dt.float32, mybir.dt.float16, mybir.dt.int16
    MULT, ADD = mybir.AluOpType.mult, mybir.AluOpType.add
    slots = len(slot_groups)
    chunks = _chunks(slot_groups)
    # chunk index that each slot belongs to
    slot_chunk = [0] * slots
    for c, (_l, _b, _n, s0, k) in enumerate(chunks):
        for j in range(k):
            slot_chunk[s0 + j] = c

    nc = bacc.Bacc("TRN2", debug=False)
    feats_d = [
        nc.dram_tensor(f"feat{l}q", [2 * S * S, 1024], f16, kind="ExternalInput")
        for l, S in enumerate(SIZES)
    ]
    idx_d = nc.dram_tensor("idx16", [128, slots * 16], i16, kind="ExternalInput")
    wts_d = nc.dram_tensor("wts", [128, slots * 8], f32, kind="ExternalInput")
    out_d = nc.dram_tensor("out", [slots, 128, 512], f16, kind="ExternalOutput")

    with ExitStack() as st:
        block = st.enter_context(nc.Block())
        itile = st.enter_context(nc.sbuf_tensor("itile", [128, slots * 16], i16))
        wtile = st.enter_context(nc.sbuf_tensor("wtile", [128, slots * 8], f32))
        gt = [
            st.enter_context(nc.sbuf_tensor(f"gt{i}", [128, 2 * CH, 1024], f16))
            for i in range(NBUF)
        ]
        ttop = [st.enter_context(nc.sbuf_tensor(f"ttop{i}", [128, 256], f32)) for i in range(2 * PIPE)]
        tbot = [st.enter_context(nc.sbuf_tensor(f"tbot{i}", [128, 256], f32)) for i in range(2 * PIPE)]
        ot = [st.enter_context(nc.sbuf_tensor(f"ot{i}", [128, 512], f16)) for i in range(PIPE_O)]
        i_sem = st.enter_context(nc.semaphore("i_sem"))
        i_sem2 = st.enter_context(nc.semaphore("i_sem2"))
        w_sem = st.enter_context(nc.semaphore("w_sem"))
        a_sem = st.enter_context(nc.semaphore("a_sem"))
        v_sem = st.enter_context(nc.semaphore("v_sem"))
        # DMA-completion sems must be per pipeline buffer: concurrent DMAs
        # incrementing one shared sem release prefix-waiters early (race).
        g_sems = [st.enter_context(nc.semaphore(f"g_sem{i}")) for i in range(NBUF)]
        o_sems = [st.enter_context(nc.semaphore(f"o_sem{i}")) for i in range(PIPE_O)]

        # split itile load: first gathers only need the first NBUF chunks' columns
        c_split = min(NBUF, len(chunks))
        split_col = slots * 16 if c_split >= len(chunks) else chunks[c_split][3] * 16

        @block.sync
        def _(eng):
            eng.dma_start(itile[:, 0:split_col], idx_d[:, 0:split_col]).then_inc(i_sem, 16)
            eng.dma_start(wtile[:], wts_d[:]).then_inc(w_sem, 16)
            if split_col < slots * 16:
                eng.dma_start(itile[:, split_col:], idx_d[:, split_col:]).then_inc(i_sem2, 16)
            for s in range(slots):
                eng.wait_ge(v_sem, 6 * (s + 1))
                eng.dma_start(out_d[s], ot[s % PIPE_O][:]).then_inc(o_sems[s % PIPE_O], 16)
            for j in range(PIPE_O):
                cnt = len(range(j, slots, PIPE_O))
                eng.wait_ge(o_sems[j], 16 * cnt)

        @block.gpsimd
        def _(eng):
            eng.load_library(library_config.mlp)
            eng.wait_ge(i_sem, 16)
            for c, (lvl, base, nrows, s0, k) in enumerate(chunks):
                if c == c_split and split_col < slots * 16:
                    eng.wait_ge(i_sem2, 16)
                if c >= NBUF:
                    # all compute on the previous occupant of this buffer done
                    _pl, _pb, _pn, ps0, pk = chunks[c - NBUF]
                    pend = ps0 + pk
                    eng.wait_ge(a_sem, 4 * pend)
                    eng.wait_ge(v_sem, 6 * pend)
                src = bass.AP(feats_d[lvl][:].tensor, base * 1024, [[1024, nrows], [1, 1024]])
                eng.dma_gather(
                    gt[c % NBUF][:, 0 : 2 * k, :], src,
                    itile[:, s0 * 16 : (s0 + k) * 16],
                    256 * k, 256 * k, 1024, elem_step=1024,
                ).then_inc(g_sems[c % NBUF], 16)
            for j in range(NBUF):
                cnt = len(range(j, len(chunks), NBUF))
                eng.wait_ge(g_sems[j], 16 * cnt)

        @block.scalar
        def _(eng):
            eng.wait_ge(w_sem, 16)
            for s in range(slots):
                c = slot_chunk[s]
                eng.wait_ge(g_sems[c % NBUF], 16 * (c // NBUF + 1))
                if s >= PIPE:
                    eng.wait_ge(v_sem, 6 * (s - PIPE + 1))
                j2 = 2 * (s - chunks[c][3])
                for b in (0, 1):
                    k = 2 * s + b
                    g00 = gt[c % NBUF][:, j2 + b, 0:256]
                    g10 = gt[c % NBUF][:, j2 + b, 512:768]
                    w0 = s * 8 + b * 4
                    eng.mul(ttop[k % (2 * PIPE)][:], g00, wtile[:, w0 : w0 + 1]).then_inc(a_sem, 1)
                    eng.mul(tbot[k % (2 * PIPE)][:], g10, wtile[:, w0 + 2 : w0 + 3]).then_inc(a_sem, 1)

        @block.vector
        def _(eng):
            for s in range(slots):
                c = slot_chunk[s]
                eng.wait_ge(g_sems[c % NBUF], 16 * (c // NBUF + 1))
                if s >= PIPE_O:
                    eng.wait_ge(o_sems[s % PIPE_O], 16 * (s // PIPE_O))
                j2 = 2 * (s - chunks[c][3])
                for b in (0, 1):
                    k = 2 * s + b
                    tt_ = ttop[k % (2 * PIPE)][:]
                    tb_ = tbot[k % (2 * PIPE)][:]
                    g01 = gt[c % NBUF][:, j2 + b, 256:512]
                    g11 = gt[c % NBUF][:, j2 + b, 768:1024]
                    w0 = s * 8 + b * 4
                    eng.wait_ge(a_sem, 2 * k + 2)
                    eng.scalar_tensor_tensor(tt_, g01, wtile[:, w0 + 1 : w0 + 2], tt_, MULT, ADD).then_inc(v_sem, 1)
                    eng.scalar_tensor_tensor(tb_, g11, wtile[:, w0 + 3 : w0 + 4], tb_, MULT, ADD).then_inc(v_sem, 1)
                    eng.tensor_add(ot[s % PIPE_O][:, b * 256 : (b + 1) * 256], tt_, tb_).then_inc(v_sem, 1)

    nc.finalize()
    return nc


def _build_quads(feat0, feat1, feat2, feat3):
    quads = []
    for lev, f in enumerate((feat0, feat1, feat2, feat3)):
        S = SIZES[lev]
        F = np.ascontiguousarray(
            np.asarray(f, np.float32).transpose(0, 2, 3, 1)
        ).reshape(-1, 256).astype(np.float16)
        TOT = F.shape[0]
        Q = np.zeros((TOT, 1024), np.float16)
        Q[:, 0:256] = F
        Q[:-1, 256:512] = F[1:]
        Q[:-S, 512:768] = F[S:]
        Q[: -S - 1, 768:1024] = F[S + 1 :]
        quads.append(Q)
    return quads


def kernel(feat0, feat1, feat2, feat3, polys, img_ids, **_kw):
    global LAST_EXEC_NS
    feats = _build_quads(feat0, feat1, feat2, feat3)
    grid_np, lvls_np = _grid_and_levels(polys)
    seg0, weights = _corners(grid_np, lvls_np, img_ids)
    groups = _build_groups(seg0, lvls_np)
    slot_groups, core_slots = _deal(groups)
    idx16, wts = _build_core_inputs(slot_groups, core_slots, seg0, weights)

    nc = _build_device(slot_groups)

    from concourse.bass_utils import run_bass_kernel_spmd

    in_maps = [
        {
            "feat0q": feats[0],
            "feat1q": feats[1],
            "feat2q": feats[2],
            "feat3q": feats[3],
            "idx16": idx16[c],
            "wts": wts[c],
        }
        for c in range(NCORES)
    ]
    res = run_bass_kernel_spmd(nc, in_maps, list(range(NCORES)), trace=_TRACE)
    LAST_EXEC_NS = res.exec_time_ns

    outbuf = np.empty((N_ROIS, HO * WO, 256), np.float32)
    for c in range(NCORES):
        o = np.asarray(res.results[c]["out"], np.float32)
        for s, (roi, dummy) in enumerate(core_slots[c]):
            if dummy:
                continue
            outbuf[roi, 0:128, :] = o[s][:, 0:256]
            outbuf[roi, 128:256, :] = o[s][:, 256:512]
    return np.ascontiguousarray(outbuf.transpose(0, 2, 1)).reshape(N_ROIS, 256, HO, WO)
